# revision 7
# baseline (speedup 1.0000x reference)
"""Causal self-attention (B=4, T=2048, C=256, H=8, HD=32) on 8 NeuronCores.

Sharding: core = 2*b + g (b = batch 0..3, g = head-group 0..1; each group =
4 heads = 128 channels). Each core: qkv projection (its slice of w_qkv),
causal softmax attention for its 4 heads, partial output projection with
w_proj[group rows, :]. Host sums the two partials per batch and adds b_proj.

v3: ScalarE-saturated pipeline with a 3+1 exp split. S and EXP run in
2-head halves (S_h01(i+1) fills its PSUM banks while EXP_b(i) reads the
other half's) so the exp stream never waits for PE. Head 3's exp is
computed on the VectorE via the Schraudolph bit trick ((A*s+B) -> int32,
bits reinterpreted as fp32), read by its AV/den matmuls as fp32 moving
data - this takes ~25% of the work off the ScalarE critical stream at
~3% elementwise error (final output err ~6e-3, gate 2e-2). AV/denominator
matmuls run as two 4-strip col-tiled waves (AV_h01+den_h23, den_h01+AV_h23)
under the EXP windows, 1 PSUM bank each (M=32 dense AV; all-ones stationary
for denominators). proj of chunk c is deferred into chunk c+1's loop
windows so chunk boundaries don't stall the exp stream. DMA layouts are
packed 2KB/partition and split across queues at startup/tail.
"""

import math

import numpy as np

B, T, C = 4, 2048, 256
H, HD = 8, 32
NCORES = 8
SCALE = 1.0 / math.sqrt(HD)
NCH = T // 512  # 4 query chunks of 512
NT = T // 128  # 16 tiles of 128

SCH_A = float(SCALE * 1.4426950408889634 * 1024.0)
SCH_B = float(15.0 * 1024.0 - 366393.0 / 8192.0)

_NC_CACHE = {}


def _build(has_bias: bool):
    import concourse.bass as bass  # noqa: F401
    import concourse.mybir as mybir
    import concourse.tile as tile
    from concourse import bacc

    F32 = mybir.dt.float32
    F16 = mybir.dt.float16
    I16 = mybir.dt.int16
    AF = mybir.ActivationFunctionType
    ALU = mybir.AluOpType

    nc = bacc.Bacc("TRN2", target_bir_lowering=False, debug=False, num_devices=NCORES)

    xT_d = nc.declare_dram_parameter("xT", [128, 2 * T], F16, isOutput=False)
    wqk_d = nc.declare_dram_parameter("wqk", [128, 512], F16, isOutput=False)
    wva_d = nc.declare_dram_parameter("wva", [128, 256], F16, isOutput=False)
    wp_d = nc.declare_dram_parameter("wp", [128, C], F16, isOutput=False)
    mk_d = nc.declare_dram_parameter("mk", [128, 512], F16, isOutput=False)
    bq_d = nc.declare_dram_parameter("bq", [128, 1], F32, isOutput=False)
    bk_d = nc.declare_dram_parameter("bk", [128, 1], F32, isOutput=False)
    bv_d = nc.declare_dram_parameter("bv", [1, 128], F32, isOutput=False)
    y_d = nc.declare_dram_parameter("y", [128, 2 * T], F16, isOutput=True)

    with tile.TileContext(nc) as tc:
        with (
            tc.tile_pool(name="cst", bufs=1) as cst,
            tc.tile_pool(name="expp", bufs=3) as expp,
            tc.tile_pool(name="schp", bufs=3) as schp,
            tc.tile_pool(name="smt", bufs=2) as smt,
            tc.tile_pool(name="yout", bufs=4) as yout,
            tc.tile_pool(name="psA", bufs=1, space="PSUM") as psA,
            tc.tile_pool(name="psB", bufs=1, space="PSUM") as psB,
            tc.tile_pool(name="psv", bufs=1, space="PSUM") as psv,
            tc.tile_pool(name="psd", bufs=1, space="PSUM") as psd,
            tc.tile_pool(name="pss", bufs=2, space="PSUM") as pss,
        ):
            # ---------------- setup ----------------
            # force the exp table load to overlap the input DMAs
            dums = cst.tile([128, 1], F32)
            dumm = cst.tile([128, 1], F16)
            nc.vector.memset(dums[:], 0.0)
            nc.scalar.activation(dumm[:], dums[:], AF.Exp, scale=1.0)

            # x^T slice 0 (4-way split) + wqk (2-way) first: they gate chunk 0
            xT = cst.tile([128, 2, T], F16)
            x0 = xT_d[:, 0:1024].rearrange("p (a t) -> p a t", a=2)
            for j in range(4):
                nc.sync.dma_start(
                    xT[:, :, 128 * j : 128 * j + 128], x0[:, :, 128 * j : 128 * j + 128]
                )
            wqk_sb = cst.tile([128, 2, 256], F16)
            for a in range(2):
                nc.sync.dma_start(
                    wqk_sb[:, a, :], wqk_d[:, 256 * a : 256 * a + 256]
                )
            wva_sb = cst.tile([128, 2, 128], F16)
            nc.sync.dma_start(wva_sb[:], wva_d[:].rearrange("p (a m) -> p a m", a=2))
            mask3 = cst.tile([128, 4, 128], F16)
            nc.sync.dma_start(mask3[:], mk_d[:].rearrange("p (h q) -> p h q", h=4))
            for cc in range(1, NCH):
                xc = xT_d[:, 1024 * cc : 1024 * cc + 1024].rearrange(
                    "p (a t) -> p a t", a=2
                )
                for j in range(2):
                    nc.sync.dma_start(
                        xT[:, :, 512 * cc + 256 * j : 512 * cc + 256 * j + 256],
                        xc[:, :, 256 * j : 256 * j + 256],
                    )
            wp_sb = cst.tile([128, 256], F16)
            nc.sync.dma_start(wp_sb[:], wp_d[:])

            ones32 = cst.tile([128, 128], F16)
            nc.vector.memset(ones32[:], 1.0)
            bco = cst.tile([128, 512], F32)
            nc.vector.memset(bco[:], SCH_B)

            if has_bias:
                bq_sb = cst.tile([128, 1], F32)
                bk_sb = cst.tile([128, 1], F32)
                nc.sync.dma_start(bq_sb[:], bq_d[:])
                nc.sync.dma_start(bk_sb[:], bk_d[:])
                bv_sb = cst.tile([1, 128], F32)
                nc.sync.dma_start(bv_sb[:], bv_d[:])
                ones1 = cst.tile([1, 128], F32)
                nc.vector.memset(ones1[:], 1.0)
                pbv = pss.tile([128, 512], F32, tag="sm")
                nc.tensor.matmul(pbv[:, :128], ones1[:], bv_sb[:], start=True, stop=True)
                bvb = cst.tile([128, 128], F32)
                nc.vector.tensor_copy(bvb[:], pbv[:, :128])
            else:
                bq_sb = bk_sb = bvb = None

            qT = cst.tile([128, T], F16)
            kT = cst.tile([128, T], F16)
            vaug = cst.tile([128, NT, 128], F16)
            otn = cst.tile([128, T], F16)

            def qk_half(n, which):
                dest, bias, wc = (
                    (qT, bq_sb, 0) if which == 0 else (kT, bk_sb, 128)
                )
                pq = pss.tile([128, 512], F32, tag="sm")
                for ko in range(2):
                    nc.tensor.matmul(
                        pq[:],
                        wqk_sb[:, ko, wc : wc + 128],
                        xT[:, ko, 512 * n : 512 * n + 512],
                        start=(ko == 0),
                        stop=(ko == 1),
                    )
                if has_bias:
                    nc.vector.tensor_scalar(
                        dest[:, 512 * n : 512 * n + 512],
                        pq[:],
                        bias[:, 0:1],
                        None,
                        ALU.add,
                    )
                else:
                    nc.vector.tensor_copy(dest[:, 512 * n : 512 * n + 512], pq[:])

            def qk_chunk(n):
                qk_half(n, 0)
                qk_half(n, 1)

            def v_tile(t):
                pv_ = pss.tile([128, 512], F32, tag="sm")
                for ko in range(2):
                    nc.tensor.matmul(
                        pv_[:, :128],
                        xT[:, ko, 128 * t : 128 * t + 128],
                        wva_sb[:, ko, :],
                        start=(ko == 0),
                        stop=(ko == 1),
                    )
                if has_bias:
                    nc.vector.tensor_tensor(
                        vaug[:, t, :], pv_[:, :128], bvb[:], ALU.add
                    )
                else:
                    nc.vector.tensor_copy(vaug[:, t, :], pv_[:, :128])

            fill_crit = []  # q/k/v for the next chunk: must flush at chunk end
            fill_lazy = []  # previous chunk's proj: consumed opportunistically

            def consume_fill():
                if fill_crit:
                    fill_crit.pop(0)()
                elif fill_lazy:
                    fill_lazy.pop(0)()

            def proj_pieces(c):
                yts = [
                    yout.tile([128, 2, 256], F16, tag="yt", name=f"yt{c}_0"),
                    yout.tile([128, 2, 256], F16, tag="yt", name=f"yt{c}_1"),
                ]

                def piece(mi):
                    def run():
                        m = 4 * c + mi
                        py = pss.tile([128, 512], F32, tag="sm")
                        nc.tensor.matmul(
                            py[:, :256],
                            otn[:, 128 * m : 128 * m + 128],
                            wp_sb[:],
                            start=True,
                            stop=True,
                        )
                        nc.vector.tensor_copy(yts[mi // 2][:, mi % 2, :], py[:, :256])
                        if c == NCH - 1:
                            nc.sync.dma_start(
                                y_d[:, 1024 * c + 256 * mi : 1024 * c + 256 * mi + 256],
                                yts[mi // 2][:, mi % 2, :],
                            )
                        elif mi % 2 == 1:
                            nc.sync.dma_start(
                                y_d[
                                    :,
                                    1024 * c
                                    + 512 * (mi // 2) : 1024 * c
                                    + 512 * (mi // 2)
                                    + 512,
                                ],
                                yts[mi // 2][:],
                            )

                    return run

                return [piece(mi) for mi in range(4)]

            # ---------------- main attention loop ----------------
            qk_chunk(0)
            v_tile(0)
            fill_crit.extend([lambda t=t: v_tile(t) for t in range(1, 4)])

            for c in range(NCH):
                ilast = 4 * c + 3
                pv = psv.tile([128, 512], F32, tag="pv")
                pd = psd.tile([128, 512], F32, tag="pd")

                def one_av(kind, h, prev):
                    i, ex, sch, off = prev
                    mov = (
                        sch[:, off:512].bitcast(F16)
                        if h == 3
                        else ex[:, h, off:512]
                    )
                    lhs = (
                        vaug[:, i, 32 * h : 32 * h + 32]
                        if kind == "av"
                        else ones32[:, 32 * h : 32 * h + 32]
                    )
                    dst = pv if kind == "av" else pd
                    nc.tensor.matmul(
                        dst[32 * h : 32 * h + 32, off:512],
                        lhs,
                        mov,
                        start=(i == 0),
                        stop=(i == ilast),
                        tile_position=(0, 32 * h),
                    )

                def wave1(prev):
                    # strips 0,1 AV (h0,h1) + strips 2,3 den (h2,h3)
                    one_av("av", 0, prev)
                    one_av("av", 1, prev)
                    one_av("den", 2, prev)
                    one_av("den", 3, prev)

                def wave2(prev):
                    one_av("den", 0, prev)
                    one_av("den", 1, prev)
                    one_av("av", 2, prev)
                    one_av("av", 3, prev)

                def s_half(i, off, sp, hs):
                    for sl, h in enumerate(hs):
                        nc.tensor.matmul(
                            sp[:, sl, off:512],
                            kT[32 * h : 32 * h + 32, 128 * i : 128 * i + 128],
                            qT[
                                32 * h : 32 * h + 32,
                                512 * c + off : 512 * c + 512,
                            ],
                            start=True,
                            stop=True,
                            tile_position=(32 * h, 0),
                        )

                if c + 1 < NCH:
                    fill_crit.append(lambda n=c + 1: qk_half(n, 0))
                    fill_crit.append(lambda n=c + 1: qk_half(n, 1))
                    fill_crit.extend(
                        [
                            lambda t=t: v_tile(t)
                            for t in range(4 * (c + 1), 4 * (c + 1) + 4)
                        ]
                    )

                prev = None
                # S_h01 is issued one iteration ahead (right after the waves,
                # before any fill pieces) so EXP_a(i+1) is never gated by
                # lower-priority PE work sitting earlier in the queue.
                spA = psA.tile([128, 2, 512], F32, tag="SA")
                s_half(0, max(0, -4 * c) * 128, spA, (0, 1))
                for i in range(0, ilast + 1):
                    off = max(0, (i - 4 * c) * 128)
                    ex = expp.tile([128, 3, 512], F16, tag="ex")
                    sch = schp.tile([128, 512], I16, tag="sch")
                    nc.scalar.activation(
                        ex[:, 0:2, off:512], spA[:, :, off:512], AF.Exp, scale=SCALE
                    )
                    spB = psB.tile([128, 2, 512], F32, tag="SB")
                    s_half(i, off, spB, (2, 3))
                    nc.scalar.activation(
                        ex[:, 2:3, off:512], spB[:, 0:1, off:512], AF.Exp, scale=SCALE
                    )
                    nc.vector.scalar_tensor_tensor(
                        sch[:, off:512],
                        spB[:, 1, off:512],
                        SCH_A,
                        bco[:, off:512],
                        ALU.mult,
                        ALU.add,
                    )
                    if i >= 4 * c:
                        nc.vector.tensor_tensor(
                            ex[:, 0:3, off : off + 128],
                            ex[:, 0:3, off : off + 128],
                            mask3[:, 0:3, :],
                            ALU.mult,
                        )
                        nc.vector.tensor_tensor(
                            sch[:, off : off + 128].bitcast(F16),
                            sch[:, off : off + 128].bitcast(F16),
                            mask3[:, 3, :],
                            ALU.mult,
                        )
                    if prev is not None:
                        wave1(prev)
                        wave2(prev)
                    if i + 1 <= ilast:
                        off2 = max(0, (i + 1 - 4 * c) * 128)
                        spA = psA.tile([128, 2, 512], F32, tag="SA")
                        s_half(i + 1, off2, spA, (0, 1))
                    if prev is not None:
                        consume_fill()
                        consume_fill()
                    prev = (i, ex, sch, off)
                wave1(prev)
                wave2(prev)
                while fill_crit:
                    fill_crit.pop(0)()

                # ---- epilogue: normalize; projection deferred into next chunk ----
                dr = smt.tile([128, 512], F32, tag="dr")
                nc.vector.reciprocal_approx_fast(dr[:], pd[:])
                nc.vector.tensor_tensor(
                    otn[:, 512 * c : 512 * c + 512], pv[:], dr[:], ALU.mult
                )
                fill_lazy.extend(proj_pieces(c))

            while fill_lazy:
                fill_lazy.pop(0)()

    nc.compile()
    return nc


def _get_nc(has_bias: bool):
    if has_bias not in _NC_CACHE:
        _NC_CACHE[has_bias] = _build(has_bias)
    return _NC_CACHE[has_bias]


_MASK = None


def _tri_mask():
    global _MASK
    if _MASK is None:
        m = np.triu(np.ones((128, 128), dtype=np.float16))  # keep key<=query
        _MASK = np.ascontiguousarray(np.tile(m, (1, 4)))  # [128, 512]
    return _MASK


def _core_inputs(core, x, w_qkv, b_qkv, w_proj):
    b, g = core // 2, core % 2
    qs, ks, vs = 128 * g, 256 + 128 * g, 512 + 128 * g
    # x^T packed as [128, c, a, 512]: row p, col block (c,a) = x[b].T[128a+p, 512c:]
    xt = np.ascontiguousarray(x[b].T).astype(np.float16)  # [256, 2048]
    xh = xt.reshape(2, 128, NCH, 512).transpose(1, 2, 0, 3).reshape(128, 2 * T)
    wqk = np.concatenate(
        [w_qkv[:, qs : qs + 128], w_qkv[:, ks : ks + 128]], axis=1
    ).astype(np.float16)  # [256, 256]
    wqkh = wqk.reshape(2, 128, 256).transpose(1, 0, 2).reshape(128, 512)
    wva = np.ascontiguousarray(w_qkv[:, vs : vs + 128]).astype(np.float16)
    wvah = wva.reshape(2, 128, 128).transpose(1, 0, 2).reshape(128, 256)
    return {
        "xT": np.ascontiguousarray(xh),
        "wqk": np.ascontiguousarray(wqkh),
        "wva": np.ascontiguousarray(wvah),
        "wp": np.ascontiguousarray(w_proj[128 * g : 128 * g + 128, :]).astype(
            np.float16
        ),
        "mk": _tri_mask(),
        "bq": np.ascontiguousarray(b_qkv[qs : qs + 128]).astype(np.float32)[:, None],
        "bk": np.ascontiguousarray(b_qkv[ks : ks + 128]).astype(np.float32)[:, None],
        "bv": np.ascontiguousarray(b_qkv[vs : vs + 128]).astype(np.float32)[None, :],
    }


def _in_maps(x, w_qkv, b_qkv, w_proj):
    return [_core_inputs(core, x, w_qkv, b_qkv, w_proj) for core in range(NCORES)]


def kernel(x, w_qkv, b_qkv, w_proj, b_proj):
    from concourse.bass_utils import run_bass_kernel_spmd

    x = np.asarray(x, dtype=np.float32)
    w_qkv = np.asarray(w_qkv, dtype=np.float32)
    b_qkv = np.asarray(b_qkv, dtype=np.float32)
    w_proj = np.asarray(w_proj, dtype=np.float32)
    b_proj = np.asarray(b_proj, dtype=np.float32)
    assert x.shape == (B, T, C), x.shape

    has_bias = bool(np.any(b_qkv))
    nc = _get_nc(has_bias)

    res = run_bass_kernel_spmd(
        nc, _in_maps(x, w_qkv, b_qkv, w_proj), list(range(NCORES))
    )
    y = np.empty((B, T, C), dtype=np.float32)
    for b in range(B):
        # y dram layout: [128, c, t, 256] -> rows 512c+128t+p
        acc = None
        for part in (res.results[2 * b]["y"], res.results[2 * b + 1]["y"]):
            yb = (
                part.astype(np.float32)
                .reshape(128, NCH, 4, 256)
                .transpose(1, 2, 0, 3)
                .reshape(T, C)
            )
            acc = yb if acc is None else acc + yb
        y[b] = acc + b_proj
    return y
